# revision 1
# baseline (speedup 1.0000x reference)
"""Causal multi-head attention (B=2, T=2048, E=1024, 16 heads) on 8 TRN2 cores.

Sharding: 8-way tensor-parallel over heads (2 heads/core) for QKV projections
and attention; one AllToAll per head re-shards the attention output over
tokens so each core computes the output projection for its 512-token block.

All matmuls run in float32r (TF32-like, full PE rate at free-dim >= 256).
The host passes x^T and the weight transposes directly as float32r inputs,
so no on-device transposes are needed except for V (computed on device).
Scores are computed transposed (S^T = K Q^T, [k-toks x q-toks]) so softmax
P^T feeds the AV matmul directly; a ones column appended to V makes the AV
matmul emit softmax denominators; causal masking is one 128x128 triangle
add per diagonal block plus block-level skipping; max-subtraction is
omitted (scores are O(1), exp cannot overflow).
"""
import sys

if "/opt/trn_rl_repo" not in sys.path:
    sys.path.insert(0, "/opt/trn_rl_repo")

import numpy as np

import concourse.bacc as bacc
import concourse.mybir as mybir
from concourse import tile
from concourse.bass_utils import run_bass_kernel_spmd

dt = mybir.dt
AF = mybir.ActivationFunctionType
ALU = mybir.AluOpType

B, T, E, HS, NH = 2, 2048, 1024, 64, 16
NCORE = 8
NTOK = B * T            # 4096
CH = 512                # token chunk
NCH = NTOK // CH        # 8
CPB = NCH // B          # chunks per batch = 4
SUB = 128
NSUB = CH // SUB        # 4
NEG = -1.0e30

_nc_cache = {}


def build_nc(stage="full"):
    nc = bacc.Bacc("TRN2", target_bir_lowering=False, debug=False,
                   num_devices=NCORE)
    f32, f32r = dt.float32, dt.float32r

    xT = nc.declare_dram_parameter("xT", [E, NTOK], f32r, isOutput=False)
    wqT = nc.declare_dram_parameter("wqT", [E, 128], f32r, isOutput=False)
    wkT = nc.declare_dram_parameter("wkT", [E, 128], f32r, isOutput=False)
    wvT = nc.declare_dram_parameter("wvT", [E, 128], f32r, isOutput=False)
    woh0 = nc.declare_dram_parameter("woh0", [512, E], f32r, isOutput=False)
    woh1 = nc.declare_dram_parameter("woh1", [512, E], f32r, isOutput=False)
    bqs = nc.declare_dram_parameter("bqs", [128, 1], f32, isOutput=False)
    bks = nc.declare_dram_parameter("bks", [128, 1], f32, isOutput=False)
    bvs = nc.declare_dram_parameter("bvs", [128, 1], f32, isOutput=False)
    bo_b = nc.declare_dram_parameter("bo_b", [128, E], f32, isOutput=False)
    eye = nc.declare_dram_parameter("eye", [128, 128], f32, isOutput=False)
    tri = nc.declare_dram_parameter("tri", [128, 128], f32, isOutput=False)
    ones_v = nc.declare_dram_parameter("ones_v", [128, NCH * NSUB], f32,
                                       isOutput=False)
    ones_r = nc.declare_dram_parameter("ones_r", [1, 64], f32, isOutput=False)
    y = nc.declare_dram_parameter("y", [CH, E], f32, isOutput=True)

    with tile.TileContext(nc) as tc:
        from contextlib import ExitStack
        with ExitStack() as top:
            const = top.enter_context(tc.tile_pool(name="const", bufs=1))
            persist = top.enter_context(tc.tile_pool(name="persist", bufs=1))
            xtp_pool = top.enter_context(tc.tile_pool(name="xtp", bufs=2))
            ps_t = top.enter_context(
                tc.tile_pool(name="ps_t", bufs=1, space="PSUM"))
            ps_q = top.enter_context(
                tc.tile_pool(name="ps_q", bufs=2, space="PSUM"))
            ps_s = top.enter_context(
                tc.tile_pool(name="ps_s", bufs=3, space="PSUM"))
            ps_a = top.enter_context(
                tc.tile_pool(name="ps_a", bufs=2, space="PSUM"))
            dram = top.enter_context(
                tc.tile_pool(name="dram", bufs=1, space="DRAM"))

            # ---- constants -------------------------------------------------
            eye_sb = const.tile([128, 128], f32, name="eye_sb")
            nc.sync.dma_start(eye_sb[:], eye[:])
            eyer_sb = const.tile([128, 128], f32r, name="eyer_sb")
            nc.vector.tensor_copy(eyer_sb[:], eye_sb[:])
            tri_sb = const.tile([128, 128], f32, name="tri_sb")
            nc.sync.dma_start(tri_sb[:], tri[:])
            onesv_sb = const.tile([128, NCH * NSUB], f32, name="onesv_sb")
            nc.sync.dma_start(onesv_sb[:], ones_v[:])
            onesr_sb = const.tile([1, 64], f32, name="onesr_sb")
            nc.sync.dma_start(onesr_sb[:], ones_r[:])
            onesr_r = const.tile([1, 64], f32r, name="onesr_r")
            nc.vector.tensor_copy(onesr_r[:], onesr_sb[:])
            bq_sb = const.tile([128, 1], f32, name="bq_sb")
            nc.sync.dma_start(bq_sb[:], bqs[:])
            bk_sb = const.tile([128, 1], f32, name="bk_sb")
            nc.sync.dma_start(bk_sb[:], bks[:])
            bv_sb = const.tile([128, 1], f32, name="bv_sb")
            nc.sync.dma_start(bv_sb[:], bvs[:])
            bo_sb = const.tile([128, E], f32, name="bo_sb")
            nc.sync.dma_start(bo_sb[:], bo_b[:])

            # ---- persistent tensors ---------------------------------------
            wq_sb = persist.tile([128, 8, 128], f32r, name="wq_sb")
            wk_sb = persist.tile([128, 8, 128], f32r, name="wk_sb")
            wv_sb = persist.tile([128, 8, 128], f32r, name="wv_sb")
            wo0_sb = persist.tile([128, 4, E], f32r, name="wo0_sb")
            wo1_sb = persist.tile([128, 4, E], f32r, name="wo1_sb")
            nc.sync.dma_start(wq_sb[:], wqT.rearrange("(e p) m -> p e m", p=128))
            nc.sync.dma_start(wk_sb[:], wkT.rearrange("(e p) m -> p e m", p=128))
            nc.sync.dma_start(wv_sb[:], wvT.rearrange("(e p) m -> p e m", p=128))
            kT = persist.tile([128, NCH, CH], f32r, name="kT")
            qT = persist.tile([128, NCH, CH], f32r, name="qT")
            vh0 = persist.tile([128, NCH * NSUB, 65], f32r, name="vh0")
            vh1 = persist.tile([128, NCH * NSUB, 65], f32r, name="vh1")

            cc_in = [dram.tile([NCH, 64, CH], f32r, name=f"cc_in{h}")
                     for h in range(2)]
            cc_out = [dram.tile([NCH, 64, CH], f32r, name=f"cc_out{h}")
                      for h in range(2)]

            # ones column of the augmented V
            nc.vector.tensor_copy(vh0[:, :, 64], onesv_sb[:])
            nc.vector.tensor_copy(vh1[:, :, 64], onesv_sb[:])

            # ---- phases B+C interleaved: QKV chunk t, then attention t-1 ---
            # B's dense QKV matmul bursts fill the PE between C's
            # scores->exp->AV chains, keeping the HAM clock warm.
            vstage = top.enter_context(tc.tile_pool(name="vstage", bufs=2))
            ppool = top.enter_context(tc.tile_pool(name="ppool", bufs=5))
            apool = top.enter_context(tc.tile_pool(name="apool", bufs=2))

            def emit_b(t):
                xTt = xtp_pool.tile([128, 8, CH], f32r, name="xTt",
                                    tag="xTt")
                for e in range(8):
                    for half in range(2):
                        nc.sync.dma_start(
                            xTt[:, e, 256 * half:256 * (half + 1)],
                            xT[128 * e:128 * (e + 1),
                               CH * t + 256 * half:CH * t + 256 * (half + 1)])

                # Q^T (scale 1/8 folded), K^T
                for wsb, bias, scale, dest in (
                        (wq_sb, bq_sb, 0.125, qT),
                        (wk_sb, bk_sb, 1.0, kT)):
                    ps = ps_q.tile([128, CH], f32, name="psqk", tag="psq")
                    for e in range(8):
                        nc.tensor.matmul(ps[:], wsb[:, e, :], xTt[:, e, :],
                                         start=(e == 0), stop=(e == 7))
                    if scale == 1.0:
                        nc.vector.tensor_scalar_add(dest[:, t, :], ps[:],
                                                    bias[:])
                    else:
                        nc.vector.tensor_scalar(
                            dest[:, t, :], ps[:], scale, bias[:],
                            ALU.mult, ALU.add)

                # V^T then transpose to V rows, split per head
                psv = ps_q.tile([128, CH], f32, name="psv", tag="psq")
                for e in range(8):
                    nc.tensor.matmul(psv[:], wv_sb[:, e, :], xTt[:, e, :],
                                     start=(e == 0), stop=(e == 7))
                vTs = vstage.tile([128, CH], f32r, name="vTs", tag="vTs")
                nc.vector.tensor_scalar_add(vTs[:], psv[:], bv_sb[:])
                for s in range(NSUB):
                    tv = ps_q.tile([128, 512], f32r, name="tpv", tag="psq")
                    nc.tensor.transpose(
                        tv[:, 0:128], vTs[:, 128 * s:128 * (s + 1)],
                        eyer_sb[:])
                    g = NSUB * t + s
                    nc.vector.tensor_copy(vh0[:, g, 0:64], tv[:, 0:64])
                    nc.vector.tensor_copy(vh1[:, g, 0:64], tv[:, 64:128])

            def emit_c(t):
                b0 = CPB * (t // CPB)
                a_pss = [ps_a.tile([128, CH], f32, name=f"a_ps{h}",
                                   tag="aps") for h in range(2)]

                def emit_scores(h, kc):
                    pb = 64 * h
                    diag = kc == t
                    pT = ppool.tile([128, NSUB, CH], f32r,
                                    name="pT", tag="pT")
                    for s in range(NSUB):
                        q0 = 128 * s if diag else 0
                        sps = ps_s.tile([128, CH], f32,
                                        name="sps", tag="sps")
                        nc.tensor.matmul(
                            sps[:, q0:CH],
                            kT[pb:pb + 64, kc, 128 * s:128 * (s + 1)],
                            qT[pb:pb + 64, t, q0:CH],
                            start=True, stop=True)
                        if diag:
                            nc.vector.tensor_add(
                                sps[:, q0:q0 + 128],
                                sps[:, q0:q0 + 128], tri_sb[:])
                        nc.scalar.activation(
                            pT[:, s, q0:CH], sps[:, q0:CH], AF.Exp)
                    return pT

                def emit_av(h, kc, pT):
                    vh = vh0 if h == 0 else vh1
                    diag = kc == t
                    for s in range(NSUB):
                        q0 = 128 * s if diag else 0
                        g = NSUB * kc + s
                        nc.tensor.matmul(
                            a_pss[h][0:65, q0:CH], vh[:, g, :],
                            pT[:, s, q0:CH],
                            start=(kc == b0 and s == 0),
                            stop=(diag and s == NSUB - 1))

                prev = None
                for kc in range(b0, t + 1):
                    pTs = (emit_scores(0, kc), emit_scores(1, kc))
                    if prev is not None:
                        emit_av(0, prev[0], prev[1][0])
                        emit_av(1, prev[0], prev[1][1])
                    prev = (kc, pTs)
                emit_av(0, prev[0], prev[1][0])
                emit_av(1, prev[0], prev[1][1])

                for h in range(2):
                    rec = apool.tile([1, CH], f32r, name="rec", tag="rec")
                    with nc.allow_low_precision(
                            reason="f32r recip feeds PE broadcast; "
                                   "psum accum stays fp32"):
                        nc.vector.reciprocal(rec[:], a_pss[h][64:65, :])
                    bc_ps = ps_t.tile([64, CH], f32, name="bc_ps",
                                      tag="pst", bufs=1)
                    nc.tensor.matmul(bc_ps[:], onesr_r[:], rec[:],
                                     start=True, stop=True)
                    bc_sb = apool.tile([64, CH], f32r, name="bc_sb",
                                       tag="bcs")
                    nc.vector.tensor_copy(bc_sb[:], bc_ps[:])
                    a_sb = apool.tile([64, CH], f32r, name="a_sb",
                                      tag="asb")
                    nc.vector.tensor_mul(a_sb[:], a_pss[h][0:64, :],
                                         bc_sb[:])
                    nc.sync.dma_start(cc_in[h][t, :, :], a_sb[:])

            for t in range(NCH):
                emit_b(t)
                if stage != "qkv" and t >= 1:
                    emit_c(t - 1)
            for r in range(4):
                nc.sync.dma_start(wo0_sb[:, r, :],
                                  woh0[128 * r:128 * (r + 1), :])
                nc.sync.dma_start(wo1_sb[:, r, :],
                                  woh1[128 * r:128 * (r + 1), :])

            if stage != "qkv":
                emit_c(NCH - 1)
                for h in range(2):
                    nc.gpsimd.collective_compute(
                        "AllToAll", ALU.bypass,
                        ins=[cc_in[h].opt()], outs=[cc_out[h].opt()],
                        replica_groups=[list(range(NCORE))])

            if stage == "qkv":
                yv = y.rearrange("(s p) e -> p s e", p=128)
                dbg = persist.tile([128, 8, CH], f32, name="dbg")
                nc.vector.tensor_copy(dbg[:], qT[:].bitcast(f32))
                nc.sync.dma_start(yv, dbg.rearrange("p c t -> p (c t)").rearrange("p (s e) -> p s e", s=4))

            if stage == "attn":
                yv = y.rearrange("(s p) e -> p s e", p=128)
                for h in range(2):
                    for c in range(NCH):
                        nc.sync.dma_start(
                            yv[64 * h:64 * (h + 1), c // 2,
                               (c % 2) * 512:(c % 2) * 512 + 512],
                            cc_in[h][c, :, :].bitcast(f32))

            # ---- phase E: output projection on this core's token block -----
            # split by head: the h0 half runs as soon as A2A#0 lands and
            # overlaps A2A#1; the h1 half adds the h0 partial and stores.
            with tc.tile_pool(name="ystage", bufs=2) as ystage:
                if stage == "full":
                    yacc = xtp_pool.tile([128, NSUB, E], f32, name="yacc",
                                         tag="xTt")
                    aTb = xtp_pool.tile([128, 2, 4, CH], f32r, name="aTb",
                                        tag="xTt")
                    aTs = [aTb[:, 0], aTb[:, 1]]
                    for h, cco in enumerate(cc_out):
                        for kt in range(8):
                            nc.sync.dma_start(
                                aTb[64 * (kt % 2):64 * (kt % 2) + 64,
                                    h, kt // 2, :],
                                cco[kt, :, :])
                    for m in range(NSUB):
                        for nch in range(2):
                            yps = ps_t.tile([128, 512], f32, name="yps",
                                            tag="pst", bufs=1)
                            for p in range(4):
                                nc.tensor.matmul(
                                    yps[:],
                                    aTs[0][:, p, 128 * m:128 * (m + 1)],
                                    wo0_sb[:, p, 512 * nch:512 * (nch + 1)],
                                    start=(p == 0), stop=(p == 3))
                            nc.vector.tensor_add(
                                yacc[:, m, 512 * nch:512 * (nch + 1)], yps[:],
                                bo_sb[:, 512 * nch:512 * (nch + 1)])
                for m in (range(NSUB) if stage == "full" else []):
                    for nch in range(2):
                        yps = ps_t.tile([128, 512], f32, name="yps",
                                        tag="pst", bufs=1)
                        for p in range(4):
                            nc.tensor.matmul(
                                yps[:], aTs[1][:, p, 128 * m:128 * (m + 1)],
                                wo1_sb[:, p, 512 * nch:512 * (nch + 1)],
                                start=(p == 0), stop=(p == 3))
                        ysb = ystage.tile([128, 512], f32, name="ysb",
                                          tag="ysb")
                        nc.vector.tensor_add(
                            ysb[:], yps[:],
                            yacc[:, m, 512 * nch:512 * (nch + 1)])
                        nc.sync.dma_start(
                            y[128 * m:128 * (m + 1),
                              512 * nch:512 * (nch + 1)],
                            ysb[:])
    nc.compile()
    return nc


def _prep_in_maps(embd_q, Wq, bq, Wk, bk, Wv, bv, Wo, bo):
    x = embd_q.reshape(NTOK, E).astype(np.float32)
    xT = np.ascontiguousarray(x.T)
    eye = np.eye(128, dtype=np.float32)
    r = np.arange(128)
    tri = np.where(r[:, None] > r[None, :], np.float32(NEG), np.float32(0.0))
    tri = np.ascontiguousarray(tri, dtype=np.float32)
    ones_v = np.ones((128, NCH * NSUB), dtype=np.float32)
    ones_r = np.ones((1, 64), dtype=np.float32)
    bo_b = np.ascontiguousarray(
        np.broadcast_to(bo.astype(np.float32), (128, E)))
    woTf = Wo.astype(np.float32).T  # [feat, out]
    # pair-interleaved per-head layouts: partition q of pair p maps to
    # feat = 128*(2p) + q  (q < 64, even kt)  or  128*(2p+1) + (q-64)
    idx = np.zeros((4, 128), dtype=np.int64)
    for p in range(4):
        idx[p, :64] = 128 * (2 * p) + np.arange(64)
        idx[p, 64:] = 128 * (2 * p + 1) + np.arange(64)
    woh0 = np.ascontiguousarray(woTf[idx.reshape(-1)])
    woh1 = np.ascontiguousarray(woTf[(idx + 64).reshape(-1)])
    in_maps = []
    for c in range(NCORE):
        sl = slice(128 * c, 128 * (c + 1))
        in_maps.append({
            "xT": xT,
            "wqT": np.ascontiguousarray(Wq[sl].astype(np.float32).T),
            "wkT": np.ascontiguousarray(Wk[sl].astype(np.float32).T),
            "wvT": np.ascontiguousarray(Wv[sl].astype(np.float32).T),
            "woh0": woh0,
            "woh1": woh1,
            "bqs": np.ascontiguousarray(
                (bq[sl] * 0.125).reshape(128, 1), dtype=np.float32),
            "bks": np.ascontiguousarray(bk[sl].reshape(128, 1),
                                        dtype=np.float32),
            "bvs": np.ascontiguousarray(bv[sl].reshape(128, 1),
                                        dtype=np.float32),
            "bo_b": bo_b,
            "eye": eye,
            "tri": tri,
            "ones_v": ones_v,
            "ones_r": ones_r,
        })
    return in_maps


def kernel(embd_q, Wq, bq, Wk, bk, Wv, bv, Wo, bo, _trace=False,
           _stage="full"):
    if _stage not in _nc_cache:
        _nc_cache[_stage] = build_nc(_stage)
    in_maps = _prep_in_maps(np.asarray(embd_q), np.asarray(Wq), np.asarray(bq),
                            np.asarray(Wk), np.asarray(bk), np.asarray(Wv),
                            np.asarray(bv), np.asarray(Wo), np.asarray(bo))
    import os
    tc_env = os.environ.get("TRACE_CORES")
    res = run_bass_kernel_spmd(
        _nc_cache[_stage], in_maps, list(range(NCORE)), trace=_trace,
        trace_cores=(list(range(NCORE)) if tc_env else None))
    out = np.concatenate(
        [res.results[c]["y"] for c in range(NCORE)], axis=0)
    out = out.reshape(B, T, E)
    kernel.last_results = res
    return out



# revision 13
# speedup vs baseline: 1.6459x; 1.6459x over previous
"""Causal multi-head attention (B=2, T=2048, E=1024, 16 heads) on 8 TRN2 cores.

Sharding: 8-way tensor-parallel over heads (2 heads/core) for QKV projections
and attention; one AllToAll per head-half re-shards the attention output over
tokens so each core computes the output projection for its 512-token block.

v2 (vs baseline):
- bf16 matmul operands everywhere (PSUM accumulation stays fp32): halves DMA
  and SBUF traffic, removes the f32r free<256 PE penalty.
- softmax normalization: reciprocal_approx_fast (DVE, ~5x faster than exact
  reciprocal) + partition_broadcast on the idle Pool engine (was: exact DVE
  reciprocal 3.3us each + PE broadcast matmul).
- causal diag masking: post-exp multiply by a 0/1 lower-tri mask on the Pool
  engine (was: -1e30 add on the DVE PSUM path before exp).
- off-diagonal exp runs as one [128,1024] activation spanning a 2-bank PSUM
  tile (halves Act instruction count on the bulk of the softmax).
- head-half 1 attention lags head-half 0 by 2 chunks; the h0 AllToAll fires
  as soon as h0 finishes and overlaps the h1 tail + output-projection loads.
- output projection accumulates both head-halves into one PSUM pass.
"""
import sys

if "/opt/trn_rl_repo" not in sys.path:
    sys.path.insert(0, "/opt/trn_rl_repo")

import numpy as np

import concourse.bacc as bacc
import concourse.mybir as mybir
from concourse import tile
from concourse.bass_utils import run_bass_kernel_spmd

dt = mybir.dt
AF = mybir.ActivationFunctionType
ALU = mybir.AluOpType

B, T, E, HS, NH = 2, 2048, 1024, 64, 16
NCORE = 8
NTOK = B * T            # 4096
CH = 512                # token chunk
NCH = NTOK // CH        # 8
CPB = NCH // B          # chunks per batch = 4
SUB = 128
NSUB = CH // SUB        # 4

_nc_cache = {}


def build_nc(_debug=False):
    nc = bacc.Bacc("TRN2", target_bir_lowering=False, debug=False,
                   num_devices=NCORE)
    f32, bf16 = dt.float32, dt.bfloat16

    dbg = {}
    if _debug:
        for nm, shp in (("d_qt", [128, CH]), ("d_kt", [128, CH]),
                        ("d_vh", [128, 130]), ("d_pt", [128, NSUB * CH]),
                        ("d_rec", [1, CH]), ("d_bc", [64, CH]),
                        ("d_asb", [64, CH]), ("d_aps", [128, CH])):
            dbg[nm] = nc.declare_dram_parameter(nm, shp, f32, isOutput=True)

    xT = nc.declare_dram_parameter("xT", [E, NTOK], bf16, isOutput=False)
    wqT = nc.declare_dram_parameter("wqT", [E, 128], bf16, isOutput=False)
    wkT = nc.declare_dram_parameter("wkT", [E, 128], bf16, isOutput=False)
    wvT = nc.declare_dram_parameter("wvT", [E, 128], bf16, isOutput=False)
    woh0 = nc.declare_dram_parameter("woh0", [512, E], bf16, isOutput=False)
    woh1 = nc.declare_dram_parameter("woh1", [512, E], bf16, isOutput=False)
    bqs = nc.declare_dram_parameter("bqs", [128, 1], f32, isOutput=False)
    bks = nc.declare_dram_parameter("bks", [128, 1], f32, isOutput=False)
    bvs = nc.declare_dram_parameter("bvs", [128, 1], f32, isOutput=False)
    bo_b = nc.declare_dram_parameter("bo_b", [128, E], f32, isOutput=False)
    eye = nc.declare_dram_parameter("eye", [128, 128], bf16, isOutput=False)
    tri01 = nc.declare_dram_parameter("tri01", [128, 128], bf16,
                                      isOutput=False)
    ones_v = nc.declare_dram_parameter("ones_v", [128, NCH * NSUB], bf16,
                                       isOutput=False)
    y = nc.declare_dram_parameter("y", [CH, E], f32, isOutput=True)

    with tile.TileContext(nc) as tc:
        from contextlib import ExitStack
        with ExitStack() as top:
            const = top.enter_context(tc.tile_pool(name="const", bufs=1))
            persist = top.enter_context(tc.tile_pool(name="persist", bufs=1))
            xtp_pool = top.enter_context(tc.tile_pool(name="xtp", bufs=2))
            vstage = top.enter_context(tc.tile_pool(name="vstage", bufs=2))
            ppool = top.enter_context(tc.tile_pool(name="ppool", bufs=4))
            apool = top.enter_context(tc.tile_pool(name="apool", bufs=2))
            bcpool = top.enter_context(tc.tile_pool(name="bcpool", bufs=2))
            recpool = top.enter_context(tc.tile_pool(name="recpool", bufs=2))
            ystage = top.enter_context(tc.tile_pool(name="ystage", bufs=2))
            dbgpool = (top.enter_context(tc.tile_pool(name="dbgp", bufs=1))
                       if _debug else None)
            ps_q = top.enter_context(
                tc.tile_pool(name="ps_q", bufs=2, space="PSUM"))
            ps_s = top.enter_context(
                tc.tile_pool(name="ps_s", bufs=2, space="PSUM"))
            ps_a = top.enter_context(
                tc.tile_pool(name="ps_a", bufs=2, space="PSUM"))
            dram = top.enter_context(
                tc.tile_pool(name="dram", bufs=1, space="DRAM"))

            # ---- persistent weights (emitted first: unblock chunk 0) ------
            wq_sb = persist.tile([128, 8, 128], bf16, name="wq_sb")
            wk_sb = persist.tile([128, 8, 128], bf16, name="wk_sb")
            wv_sb = persist.tile([128, 8, 128], bf16, name="wv_sb")
            nc.sync.dma_start(wq_sb[:], wqT.rearrange("(e p) m -> p e m", p=128))
            nc.sync.dma_start(wk_sb[:], wkT.rearrange("(e p) m -> p e m", p=128))
            nc.sync.dma_start(wv_sb[:], wvT.rearrange("(e p) m -> p e m", p=128))

            # ---- constants ------------------------------------------------
            eye_sb = const.tile([128, 128], bf16, name="eye_sb")
            nc.sync.dma_start(eye_sb[:], eye[:])
            tri_sb = const.tile([128, 128], bf16, name="tri_sb")
            nc.sync.dma_start(tri_sb[:], tri01[:])
            onesv_sb = const.tile([128, NCH * NSUB], bf16, name="onesv_sb")
            nc.sync.dma_start(onesv_sb[:], ones_v[:])
            bq_sb = const.tile([128, 1], f32, name="bq_sb")
            nc.sync.dma_start(bq_sb[:], bqs[:])
            bk_sb = const.tile([128, 1], f32, name="bk_sb")
            nc.sync.dma_start(bk_sb[:], bks[:])
            bv_sb = const.tile([128, 1], f32, name="bv_sb")
            nc.sync.dma_start(bv_sb[:], bvs[:])
            bo_sb = const.tile([128, E], f32, name="bo_sb")
            nc.sync.dma_start(bo_sb[:], bo_b[:])

            # ---- persistent activations -----------------------------------
            kT = persist.tile([128, NCH, CH], bf16, name="kT")
            qT = persist.tile([128, NCH, CH], bf16, name="qT")
            # V rows per k-token group g; cols 0:64 = h0 feats, 64 = ones,
            # 65:129 = h1 feats, 129 = ones.  AV stationary h = [:, g,
            # 65h:65h+65]; the ones row makes the AV matmul emit softmax
            # denominators in PSUM row 64.
            vh = persist.tile([128, NCH * NSUB, 130], bf16, name="vh")
            nc.vector.tensor_copy(vh[:, :, 64], onesv_sb[:])
            nc.vector.tensor_copy(vh[:, :, 129], onesv_sb[:])

            wo0_sb = persist.tile([128, 4, E], bf16, name="wo0_sb")
            wo1_sb = persist.tile([128, 4, E], bf16, name="wo1_sb")

            cc_in = [dram.tile([NCH, 64, CH], bf16, name=f"cc_in{h}")
                     for h in range(2)]
            cc_out = [dram.tile([NCH, 64, CH], bf16, name=f"cc_out{h}")
                      for h in range(2)]

            # ---- phase B: QKV projection for one token chunk ---------------
            def emit_b(t):
                xTt = xtp_pool.tile([128, 8, CH], bf16, name="xTt", tag="xTt")
                for e in range(8):
                    nc.sync.dma_start(
                        xTt[:, e, :],
                        xT[128 * e:128 * (e + 1), CH * t:CH * (t + 1)])
                for wsb, bias, scale, dest in (
                        (wq_sb, bq_sb, 0.125, qT),
                        (wk_sb, bk_sb, None, kT)):
                    ps = ps_q.tile([128, CH], f32, name="psqk", tag="psq")
                    for e in range(8):
                        nc.tensor.matmul(ps[:], wsb[:, e, :], xTt[:, e, :],
                                         start=(e == 0), stop=(e == 7))
                    if scale is None:
                        nc.vector.tensor_scalar_add(dest[:, t, :], ps[:],
                                                    bias[:])
                    else:
                        nc.vector.tensor_scalar(
                            dest[:, t, :], ps[:], scale, bias[:],
                            ALU.mult, ALU.add)

                psv = ps_q.tile([128, CH], f32, name="psv", tag="psq")
                for e in range(8):
                    nc.tensor.matmul(psv[:], wv_sb[:, e, :], xTt[:, e, :],
                                     start=(e == 0), stop=(e == 7))
                vTs = vstage.tile([128, CH], bf16, name="vTs", tag="vTs")
                nc.vector.tensor_scalar_add(vTs[:], psv[:], bv_sb[:])
                for s in range(NSUB):
                    tv = ps_q.tile([128, 128], bf16, name="tv", tag="psq")
                    nc.tensor.transpose(
                        tv[:], vTs[:, 128 * s:128 * (s + 1)], eye_sb[:])
                    g = NSUB * t + s
                    nc.vector.tensor_copy(vh[:, g, 0:64], tv[:, 0:64])
                    nc.vector.tensor_copy(vh[:, g, 65:129], tv[:, 64:128])

            # ---- phase C: attention for one (chunk, head-half) -------------
            def emit_c(t, h):
                b0 = CPB * (t // CPB)
                pb = 64 * h
                a_ps = ps_a.tile([128, CH], f32, name="a_ps", tag="aps")

                def emit_scores(kc):
                    diag = kc == t
                    pT = ppool.tile([128, NSUB, CH], bf16, name="pT",
                                    tag="pT")
                    for j in range(2):
                        sp = ps_s.tile([128, 2 * CH], f32, name="sp",
                                       tag="sps")
                        for jj in range(2):
                            s = 2 * j + jj
                            q0 = SUB * s if diag else 0
                            nc.tensor.matmul(
                                sp[:, CH * jj + q0:CH * jj + CH],
                                kT[pb:pb + 64, kc, SUB * s:SUB * (s + 1)],
                                qT[pb:pb + 64, t, q0:CH],
                                start=True, stop=True)
                        if diag:
                            for jj in range(2):
                                s = 2 * j + jj
                                q0 = SUB * s
                                nc.scalar.activation(
                                    pT[:, s, q0:CH],
                                    sp[:, CH * jj + q0:CH * jj + CH], AF.Exp)
                                nc.vector.tensor_mul(
                                    pT[:, s, q0:q0 + SUB],
                                    pT[:, s, q0:q0 + SUB], tri_sb[:])
                        else:
                            nc.scalar.activation(
                                pT[:, 2 * j:2 * j + 2, :], sp[:], AF.Exp)
                    return pT

                def emit_av(kc, pT):
                    diag = kc == t
                    for s in range(NSUB):
                        q0 = SUB * s if diag else 0
                        g = NSUB * kc + s
                        nc.tensor.matmul(
                            a_ps[0:65, q0:CH], vh[:, g, 65 * h:65 * h + 65],
                            pT[:, s, q0:CH],
                            start=(kc == b0 and s == 0),
                            stop=(diag and s == NSUB - 1))

                prev = None
                for kc in range(b0, t + 1):
                    pT = emit_scores(kc)
                    if prev is not None:
                        emit_av(*prev)
                    prev = (kc, pT)
                emit_av(*prev)

                den = recpool.tile([1, CH], f32, name="den", tag="den")
                nc.vector.tensor_copy(den[:], a_ps[64:65, :])
                rec = recpool.tile([1, CH], f32, name="rec", tag="rec")
                nc.vector.reciprocal_approx_fast(out=rec[:], in_=den[:])
                bc = bcpool.tile([64, CH], f32, name="bc", tag="bc")
                nc.gpsimd.partition_broadcast(bc[:], rec[:])
                a_sb = apool.tile([64, CH], bf16, name="a_sb", tag="asb")
                nc.vector.tensor_mul(a_sb[:], a_ps[0:64, :], bc[:])
                nc.sync.dma_start(cc_in[h][t, :, :], a_sb[:])

                if _debug and t == 0 and h == 0:
                    dpt = dbgpool.tile([128, NSUB * CH], f32, name="dpt")
                    nc.vector.tensor_copy(
                        dpt[:], prev[1][:].rearrange("p s c -> p (s c)"))
                    nc.sync.dma_start(dbg["d_pt"][:], dpt[:])
                    daps = dbgpool.tile([128, CH], f32, name="daps")
                    nc.vector.tensor_copy(daps[:], a_ps[:])
                    nc.sync.dma_start(dbg["d_aps"][:], daps[:])
                    drec = dbgpool.tile([1, CH], f32, name="drec")
                    nc.vector.tensor_copy(drec[:], rec[:])
                    nc.sync.dma_start(dbg["d_rec"][:], drec[:])
                    dbc = dbgpool.tile([64, CH], f32, name="dbc")
                    nc.vector.tensor_copy(dbc[:], bc[:])
                    nc.sync.dma_start(dbg["d_bc"][:], dbc[:])
                    dasb = dbgpool.tile([64, CH], f32, name="dasb")
                    nc.vector.tensor_copy(dasb[:], a_sb[:])
                    nc.sync.dma_start(dbg["d_asb"][:], dasb[:])

            # ---- main pipeline: QKV(t) | h0-attn(t-1) | h1-attn(t-3) ------
            for t in range(NCH):
                emit_b(t)
                if _debug and t == 0:
                    dqt = dbgpool.tile([128, CH], f32, name="dqt")
                    nc.vector.tensor_copy(dqt[:], qT[:, 0, :])
                    nc.sync.dma_start(dbg["d_qt"][:], dqt[:])
                    dkt = dbgpool.tile([128, CH], f32, name="dkt")
                    nc.vector.tensor_copy(dkt[:], kT[:, 0, :])
                    nc.sync.dma_start(dbg["d_kt"][:], dkt[:])
                    dvh = dbgpool.tile([128, 130], f32, name="dvh")
                    nc.vector.tensor_copy(dvh[:], vh[:, 0, :])
                    nc.sync.dma_start(dbg["d_vh"][:], dvh[:])
                if t >= 1:
                    emit_c(t - 1, 0)
                if t >= 3:
                    emit_c(t - 3, 1)
            # wo weights: DMA-idle window once all xT chunks are in flight
            for r in range(4):
                nc.sync.dma_start(wo0_sb[:, r, :],
                                  woh0[128 * r:128 * (r + 1), :])
                nc.sync.dma_start(wo1_sb[:, r, :],
                                  woh1[128 * r:128 * (r + 1), :])

            emit_c(NCH - 1, 0)
            emit_c(NCH - 3, 1)
            nc.gpsimd.collective_compute(
                "AllToAll", ALU.bypass,
                ins=[cc_in[0].opt()], outs=[cc_out[0].opt()],
                replica_groups=[list(range(NCORE))])

            aTb = xtp_pool.tile([128, 2, 4, CH], bf16, name="aTb", tag="xTt")
            for kt in range(8):
                nc.sync.dma_start(
                    aTb[64 * (kt % 2):64 * (kt % 2) + 64, 0, kt // 2, :],
                    cc_out[0][kt, :, :])

            emit_c(NCH - 2, 1)
            emit_c(NCH - 1, 1)
            nc.gpsimd.collective_compute(
                "AllToAll", ALU.bypass,
                ins=[cc_in[1].opt()], outs=[cc_out[1].opt()],
                replica_groups=[list(range(NCORE))])
            for kt in range(8):
                nc.sync.dma_start(
                    aTb[64 * (kt % 2):64 * (kt % 2) + 64, 1, kt // 2, :],
                    cc_out[1][kt, :, :])

            # ---- phase E: output projection on this core's token block ----
            # h0 partials (deps ready before A2A#1 lands) overlap A2A#1.
            wo_sbs = (wo0_sb, wo1_sb)
            for m in range(NSUB):
                for nchk in range(2):
                    yps = ps_q.tile([128, CH], f32, name="yps", tag="psq")
                    for h in range(2):
                        for p in range(4):
                            nc.tensor.matmul(
                                yps[:],
                                aTb[:, h, p, SUB * m:SUB * (m + 1)],
                                wo_sbs[h][:, p, CH * nchk:CH * (nchk + 1)],
                                start=(h == 0 and p == 0),
                                stop=(h == 1 and p == 3))
                    ysb = ystage.tile([128, CH], f32, name="ysb", tag="ysb")
                    nc.vector.tensor_add(
                        ysb[:], yps[:], bo_sb[:, CH * nchk:CH * (nchk + 1)])
                    nc.sync.dma_start(
                        y[SUB * m:SUB * (m + 1),
                          CH * nchk:CH * (nchk + 1)],
                        ysb[:])
    nc.compile()
    return nc


def _prep_in_maps(embd_q, Wq, bq, Wk, bk, Wv, bv, Wo, bo):
    import ml_dtypes
    bf16 = ml_dtypes.bfloat16
    x = embd_q.reshape(NTOK, E).astype(np.float32)
    xT = np.ascontiguousarray(x.T.astype(bf16))
    eye = np.eye(128, dtype=bf16)
    r = np.arange(128)
    # pT is [k-part, q-col]; mask out k > q (future tokens)
    tri01 = np.ascontiguousarray(
        np.where(r[:, None] > r[None, :], 0.0, 1.0).astype(bf16))
    ones_v = np.ones((128, NCH * NSUB), dtype=bf16)
    bo_b = np.ascontiguousarray(
        np.broadcast_to(bo.astype(np.float32), (128, E)))
    woTf = Wo.astype(np.float32).T  # [feat, out]
    # pair-interleaved per-head layouts: partition q of pair p maps to
    # feat = 128*(2p) + q  (q < 64, even kt)  or  128*(2p+1) + (q-64)
    idx = np.zeros((4, 128), dtype=np.int64)
    for p in range(4):
        idx[p, :64] = 128 * (2 * p) + np.arange(64)
        idx[p, 64:] = 128 * (2 * p + 1) + np.arange(64)
    woh0 = np.ascontiguousarray(woTf[idx.reshape(-1)].astype(bf16))
    woh1 = np.ascontiguousarray(woTf[(idx + 64).reshape(-1)].astype(bf16))
    in_maps = []
    for c in range(NCORE):
        sl = slice(128 * c, 128 * (c + 1))
        in_maps.append({
            "xT": xT,
            "wqT": np.ascontiguousarray(Wq[sl].astype(np.float32).T.astype(bf16)),
            "wkT": np.ascontiguousarray(Wk[sl].astype(np.float32).T.astype(bf16)),
            "wvT": np.ascontiguousarray(Wv[sl].astype(np.float32).T.astype(bf16)),
            "woh0": woh0,
            "woh1": woh1,
            "bqs": np.ascontiguousarray(
                (bq[sl] * 0.125).reshape(128, 1), dtype=np.float32),
            "bks": np.ascontiguousarray(bk[sl].reshape(128, 1),
                                        dtype=np.float32),
            "bvs": np.ascontiguousarray(bv[sl].reshape(128, 1),
                                        dtype=np.float32),
            "bo_b": bo_b,
            "eye": eye,
            "tri01": tri01,
            "ones_v": ones_v,
        })
    return in_maps


def kernel(embd_q, Wq, bq, Wk, bk, Wv, bv, Wo, bo, _trace=False,
           _debug=False):
    key = ("dbg" if _debug else "nc")
    if key not in _nc_cache:
        _nc_cache[key] = build_nc(_debug=_debug)
    in_maps = _prep_in_maps(np.asarray(embd_q), np.asarray(Wq), np.asarray(bq),
                            np.asarray(Wk), np.asarray(bk), np.asarray(Wv),
                            np.asarray(bv), np.asarray(Wo), np.asarray(bo))
    import os
    tc_env = os.environ.get("TRACE_CORES")
    res = run_bass_kernel_spmd(
        _nc_cache[key], in_maps, list(range(NCORE)), trace=_trace,
        trace_cores=(list(range(NCORE)) if tc_env else None))
    out = np.concatenate(
        [res.results[c]["y"] for c in range(NCORE)], axis=0)
    out = out.reshape(B, T, E)
    kernel.last_results = res
    return out


# revision 16
# speedup vs baseline: 1.6883x; 1.0258x over previous
"""Causal multi-head attention (B=2, T=2048, E=1024, 16 heads) on 8 TRN2 cores.

Sharding: 8-way tensor-parallel over heads (2 heads/core) for QKV projections
and attention; one AllToAll per head-half re-shards the attention output over
tokens so each core computes the output projection for its 512-token block.

v2 (vs baseline):
- bf16 matmul operands everywhere (PSUM accumulation stays fp32): halves DMA
  and SBUF traffic, removes the f32r free<256 PE penalty.
- softmax normalization: reciprocal_approx_fast (DVE, ~5x faster than exact
  reciprocal) + partition_broadcast on the idle Pool engine (was: exact DVE
  reciprocal 3.3us each + PE broadcast matmul).
- causal diag masking: post-exp multiply by a 0/1 lower-tri mask on the Pool
  engine (was: -1e30 add on the DVE PSUM path before exp).
- off-diagonal exp runs as one [128,1024] activation spanning a 2-bank PSUM
  tile (halves Act instruction count on the bulk of the softmax).
- head-half 1 attention lags head-half 0 by 2 chunks; the h0 AllToAll fires
  as soon as h0 finishes and overlaps the h1 tail + output-projection loads.
- output projection accumulates both head-halves into one PSUM pass.
"""
import sys

if "/opt/trn_rl_repo" not in sys.path:
    sys.path.insert(0, "/opt/trn_rl_repo")

import numpy as np

import concourse.bacc as bacc
import concourse.mybir as mybir
from concourse import tile
from concourse.bass_utils import run_bass_kernel_spmd

dt = mybir.dt
AF = mybir.ActivationFunctionType
ALU = mybir.AluOpType

B, T, E, HS, NH = 2, 2048, 1024, 64, 16
NCORE = 8
NTOK = B * T            # 4096
CH = 512                # token chunk
NCH = NTOK // CH        # 8
CPB = NCH // B          # chunks per batch = 4
SUB = 128
NSUB = CH // SUB        # 4

_nc_cache = {}


def build_nc(_debug=False):
    nc = bacc.Bacc("TRN2", target_bir_lowering=False, debug=False,
                   num_devices=NCORE)
    f32, bf16 = dt.float32, dt.bfloat16

    dbg = {}
    if _debug:
        for nm, shp in (("d_qt", [128, CH]), ("d_kt", [128, CH]),
                        ("d_vh", [128, 130]), ("d_pt", [128, NSUB * CH]),
                        ("d_rec", [1, CH]), ("d_bc", [64, CH]),
                        ("d_asb", [64, CH]), ("d_aps", [128, CH])):
            dbg[nm] = nc.declare_dram_parameter(nm, shp, f32, isOutput=True)

    xT = nc.declare_dram_parameter("xT", [E, NTOK], bf16, isOutput=False)
    wqT = nc.declare_dram_parameter("wqT", [E, 128], bf16, isOutput=False)
    wkT = nc.declare_dram_parameter("wkT", [E, 128], bf16, isOutput=False)
    wvT = nc.declare_dram_parameter("wvT", [E, 128], bf16, isOutput=False)
    woh0 = nc.declare_dram_parameter("woh0", [512, E], bf16, isOutput=False)
    woh1 = nc.declare_dram_parameter("woh1", [512, E], bf16, isOutput=False)
    bqs = nc.declare_dram_parameter("bqs", [128, 1], f32, isOutput=False)
    bks = nc.declare_dram_parameter("bks", [128, 1], f32, isOutput=False)
    bvs = nc.declare_dram_parameter("bvs", [128, 1], f32, isOutput=False)
    bo_b = nc.declare_dram_parameter("bo_b", [128, E], f32, isOutput=False)
    eye = nc.declare_dram_parameter("eye", [128, 128], bf16, isOutput=False)
    tri01 = nc.declare_dram_parameter("tri01", [128, 128], bf16,
                                      isOutput=False)
    ones_v = nc.declare_dram_parameter("ones_v", [128, NCH * NSUB], bf16,
                                       isOutput=False)
    y = nc.declare_dram_parameter("y", [CH, E], f32, isOutput=True)

    with tile.TileContext(nc) as tc:
        from contextlib import ExitStack
        with ExitStack() as top:
            const = top.enter_context(tc.tile_pool(name="const", bufs=1))
            persist = top.enter_context(tc.tile_pool(name="persist", bufs=1))
            xtp_pool = top.enter_context(tc.tile_pool(name="xtp", bufs=2))
            vstage = top.enter_context(tc.tile_pool(name="vstage", bufs=2))
            ppool = top.enter_context(tc.tile_pool(name="ppool", bufs=4))
            apool = top.enter_context(tc.tile_pool(name="apool", bufs=2))
            bcpool = top.enter_context(tc.tile_pool(name="bcpool", bufs=2))
            recpool = top.enter_context(tc.tile_pool(name="recpool", bufs=2))
            ystage = top.enter_context(tc.tile_pool(name="ystage", bufs=2))
            dbgpool = (top.enter_context(tc.tile_pool(name="dbgp", bufs=1))
                       if _debug else None)
            ps_q = top.enter_context(
                tc.tile_pool(name="ps_q", bufs=2, space="PSUM"))
            ps_s = top.enter_context(
                tc.tile_pool(name="ps_s", bufs=2, space="PSUM"))
            ps_a = top.enter_context(
                tc.tile_pool(name="ps_a", bufs=2, space="PSUM"))
            dram = top.enter_context(
                tc.tile_pool(name="dram", bufs=1, space="DRAM"))

            # ---- persistent weights (emitted first: unblock chunk 0) ------
            wq_sb = persist.tile([128, 8, 128], bf16, name="wq_sb")
            wk_sb = persist.tile([128, 8, 128], bf16, name="wk_sb")
            wv_sb = persist.tile([128, 8, 128], bf16, name="wv_sb")
            nc.sync.dma_start(wq_sb[:], wqT.rearrange("(e p) m -> p e m", p=128))
            nc.sync.dma_start(wk_sb[:], wkT.rearrange("(e p) m -> p e m", p=128))
            nc.sync.dma_start(wv_sb[:], wvT.rearrange("(e p) m -> p e m", p=128))

            # ---- constants ------------------------------------------------
            eye_sb = const.tile([128, 128], bf16, name="eye_sb")
            nc.sync.dma_start(eye_sb[:], eye[:])
            tri_sb = const.tile([128, 128], bf16, name="tri_sb")
            nc.sync.dma_start(tri_sb[:], tri01[:])
            onesv_sb = const.tile([128, NCH * NSUB], bf16, name="onesv_sb")
            nc.sync.dma_start(onesv_sb[:], ones_v[:])
            bq_sb = const.tile([128, 1], f32, name="bq_sb")
            nc.sync.dma_start(bq_sb[:], bqs[:])
            bk_sb = const.tile([128, 1], f32, name="bk_sb")
            nc.sync.dma_start(bk_sb[:], bks[:])
            bv_sb = const.tile([128, 1], f32, name="bv_sb")
            nc.sync.dma_start(bv_sb[:], bvs[:])
            bo_sb = const.tile([128, E], f32, name="bo_sb")
            nc.sync.dma_start(bo_sb[:], bo_b[:])

            # ---- persistent activations -----------------------------------
            kT = persist.tile([128, NCH, CH], bf16, name="kT")
            qT = persist.tile([128, NCH, CH], bf16, name="qT")
            # V rows per k-token group g; cols 0:64 = h0 feats, 64 = ones,
            # 65:129 = h1 feats, 129 = ones.  AV stationary h = [:, g,
            # 65h:65h+65]; the ones row makes the AV matmul emit softmax
            # denominators in PSUM row 64.
            vh = persist.tile([128, NCH * NSUB, 130], bf16, name="vh")
            nc.vector.tensor_copy(vh[:, :, 64], onesv_sb[:])
            nc.vector.tensor_copy(vh[:, :, 129], onesv_sb[:])

            wo0_sb = persist.tile([128, 4, E], bf16, name="wo0_sb")
            wo1_sb = persist.tile([128, 4, E], bf16, name="wo1_sb")

            cc_in = [dram.tile([NCH, 64, CH], bf16, name=f"cc_in{h}")
                     for h in range(2)]
            cc_out = [dram.tile([NCH, 64, CH], bf16, name=f"cc_out{h}")
                      for h in range(2)]

            xTr = xT.rearrange("(e p) n -> p e n", p=128)

            # ---- phase B: QKV projection for one token chunk ---------------
            def emit_b(t):
                xTt = xtp_pool.tile([128, 8, CH], bf16, name="xTt", tag="xTt")
                # single DMA -> one sem covers all 24 projection matmuls, so
                # the accumulation chains pipeline without per-matmul waits
                nc.sync.dma_start(xTt[:], xTr[:, :, CH * t:CH * (t + 1)])
                for wsb, bias, scale, dest in (
                        (wq_sb, bq_sb, 0.125, qT),
                        (wk_sb, bk_sb, None, kT)):
                    ps = ps_q.tile([128, CH], f32, name="psqk", tag="psq")
                    for e in range(8):
                        nc.tensor.matmul(ps[:], wsb[:, e, :], xTt[:, e, :],
                                         start=(e == 0), stop=(e == 7))
                    if scale is None:
                        nc.vector.tensor_scalar_add(dest[:, t, :], ps[:],
                                                    bias[:])
                    else:
                        nc.vector.tensor_scalar(
                            dest[:, t, :], ps[:], scale, bias[:],
                            ALU.mult, ALU.add)

                psv = ps_q.tile([128, CH], f32, name="psv", tag="psq")
                for e in range(8):
                    nc.tensor.matmul(psv[:], wv_sb[:, e, :], xTt[:, e, :],
                                     start=(e == 0), stop=(e == 7))
                vTs = vstage.tile([128, CH], bf16, name="vTs", tag="vTs")
                nc.vector.tensor_scalar_add(vTs[:], psv[:], bv_sb[:])
                for s in range(NSUB):
                    tv = ps_q.tile([128, 128], bf16, name="tv", tag="psq")
                    nc.tensor.transpose(
                        tv[:], vTs[:, 128 * s:128 * (s + 1)], eye_sb[:])
                    g = NSUB * t + s
                    nc.vector.tensor_copy(vh[:, g, 0:64], tv[:, 0:64])
                    nc.vector.tensor_copy(vh[:, g, 65:129], tv[:, 64:128])

            # ---- phase C: attention for one (chunk, head-half) -------------
            def emit_c(t, h):
                b0 = CPB * (t // CPB)
                pb = 64 * h
                a_ps = ps_a.tile([128, CH], f32, name="a_ps", tag="aps")

                def emit_scores(kc):
                    diag = kc == t
                    pT = ppool.tile([128, NSUB, CH], bf16, name="pT",
                                    tag="pT")
                    for j in range(2):
                        sp = ps_s.tile([128, 2 * CH], f32, name="sp",
                                       tag="sps")
                        for jj in range(2):
                            s = 2 * j + jj
                            q0 = SUB * s if diag else 0
                            nc.tensor.matmul(
                                sp[:, CH * jj + q0:CH * jj + CH],
                                kT[pb:pb + 64, kc, SUB * s:SUB * (s + 1)],
                                qT[pb:pb + 64, t, q0:CH],
                                start=True, stop=True)
                        if diag:
                            for jj in range(2):
                                s = 2 * j + jj
                                q0 = SUB * s
                                nc.scalar.activation(
                                    pT[:, s, q0:CH],
                                    sp[:, CH * jj + q0:CH * jj + CH], AF.Exp)
                                nc.vector.tensor_mul(
                                    pT[:, s, q0:q0 + SUB],
                                    pT[:, s, q0:q0 + SUB], tri_sb[:])
                        else:
                            nc.scalar.activation(
                                pT[:, 2 * j:2 * j + 2, :], sp[:], AF.Exp)
                    return pT

                def emit_av(kc, pT):
                    diag = kc == t
                    for s in range(NSUB):
                        q0 = SUB * s if diag else 0
                        g = NSUB * kc + s
                        nc.tensor.matmul(
                            a_ps[0:65, q0:CH], vh[:, g, 65 * h:65 * h + 65],
                            pT[:, s, q0:CH],
                            start=(kc == b0 and s == 0),
                            stop=(diag and s == NSUB - 1))

                prev = None
                for kc in range(b0, t + 1):
                    pT = emit_scores(kc)
                    if prev is not None:
                        emit_av(*prev)
                    prev = (kc, pT)
                emit_av(*prev)

                den = recpool.tile([1, CH], f32, name="den", tag="den")
                nc.vector.tensor_copy(den[:], a_ps[64:65, :])
                rec = recpool.tile([1, CH], f32, name="rec", tag="rec")
                nc.vector.reciprocal_approx_fast(out=rec[:], in_=den[:])
                bc = bcpool.tile([64, CH], f32, name="bc", tag="bc")
                nc.gpsimd.partition_broadcast(bc[:], rec[:])
                a_sb = apool.tile([64, CH], bf16, name="a_sb", tag="asb")
                nc.vector.tensor_mul(a_sb[:], a_ps[0:64, :], bc[:])
                nc.sync.dma_start(cc_in[h][t, :, :], a_sb[:])

                if _debug and t == 0 and h == 0:
                    dpt = dbgpool.tile([128, NSUB * CH], f32, name="dpt")
                    nc.vector.tensor_copy(
                        dpt[:], prev[1][:].rearrange("p s c -> p (s c)"))
                    nc.sync.dma_start(dbg["d_pt"][:], dpt[:])
                    daps = dbgpool.tile([128, CH], f32, name="daps")
                    nc.vector.tensor_copy(daps[:], a_ps[:])
                    nc.sync.dma_start(dbg["d_aps"][:], daps[:])
                    drec = dbgpool.tile([1, CH], f32, name="drec")
                    nc.vector.tensor_copy(drec[:], rec[:])
                    nc.sync.dma_start(dbg["d_rec"][:], drec[:])
                    dbc = dbgpool.tile([64, CH], f32, name="dbc")
                    nc.vector.tensor_copy(dbc[:], bc[:])
                    nc.sync.dma_start(dbg["d_bc"][:], dbc[:])
                    dasb = dbgpool.tile([64, CH], f32, name="dasb")
                    nc.vector.tensor_copy(dasb[:], a_sb[:])
                    nc.sync.dma_start(dbg["d_asb"][:], dasb[:])

            # ---- main pipeline: QKV(t) | h0-attn(t-1) | h1-attn(t-2) ------
            for t in range(NCH):
                emit_b(t)
                if _debug and t == 0:
                    dqt = dbgpool.tile([128, CH], f32, name="dqt")
                    nc.vector.tensor_copy(dqt[:], qT[:, 0, :])
                    nc.sync.dma_start(dbg["d_qt"][:], dqt[:])
                    dkt = dbgpool.tile([128, CH], f32, name="dkt")
                    nc.vector.tensor_copy(dkt[:], kT[:, 0, :])
                    nc.sync.dma_start(dbg["d_kt"][:], dkt[:])
                    dvh = dbgpool.tile([128, 130], f32, name="dvh")
                    nc.vector.tensor_copy(dvh[:], vh[:, 0, :])
                    nc.sync.dma_start(dbg["d_vh"][:], dvh[:])
                if t >= 1:
                    emit_c(t - 1, 0)
                if t >= 2:
                    emit_c(t - 2, 1)
            # wo weights: DMA-idle window once all xT chunks are in flight
            nc.sync.dma_start(wo0_sb[:],
                              woh0.rearrange("(r p) e -> p r e", p=128))
            nc.sync.dma_start(wo1_sb[:],
                              woh1.rearrange("(r p) e -> p r e", p=128))

            emit_c(NCH - 1, 0)
            nc.gpsimd.collective_compute(
                "AllToAll", ALU.bypass,
                ins=[cc_in[0].opt()], outs=[cc_out[0].opt()],
                replica_groups=[list(range(NCORE))])

            aTb = xtp_pool.tile([128, 2, 4, CH], bf16, name="aTb", tag="xTt")
            nc.sync.dma_start(
                aTb[:, 0],
                cc_out[0].rearrange("(a two) f n -> (two f) a n", two=2))

            emit_c(NCH - 2, 1)

            # ---- phase E0: h0 half of the output projection ---------------
            # (deps: A2A#0 + wo0 only — fills PE while h1 tail + A2A#1 run)
            yacc = persist.tile([128, NSUB, E], f32, name="yacc")
            def emit_y0():
                for m in range(NSUB):
                    for nchk in range(2):
                        yps = ps_q.tile([128, CH], f32, name="yps", tag="psq")
                        for p in range(4):
                            nc.tensor.matmul(
                                yps[:],
                                aTb[:, 0, p, SUB * m:SUB * (m + 1)],
                                wo0_sb[:, p, CH * nchk:CH * (nchk + 1)],
                                start=(p == 0), stop=(p == 3))
                        nc.vector.tensor_add(
                            yacc[:, m, CH * nchk:CH * (nchk + 1)], yps[:],
                            bo_sb[:, CH * nchk:CH * (nchk + 1)])

            emit_c(NCH - 1, 1)
            nc.gpsimd.collective_compute(
                "AllToAll", ALU.bypass,
                ins=[cc_in[1].opt()], outs=[cc_out[1].opt()],
                replica_groups=[list(range(NCORE))])
            emit_y0()
            nc.sync.dma_start(
                aTb[:, 1],
                cc_out[1].rearrange("(a two) f n -> (two f) a n", two=2))

            # ---- phase E1: h1 half + store --------------------------------
            for m in range(NSUB):
                for nchk in range(2):
                    yps = ps_q.tile([128, CH], f32, name="yps", tag="psq")
                    for p in range(4):
                        nc.tensor.matmul(
                            yps[:],
                            aTb[:, 1, p, SUB * m:SUB * (m + 1)],
                            wo1_sb[:, p, CH * nchk:CH * (nchk + 1)],
                            start=(p == 0), stop=(p == 3))
                    ysb = ystage.tile([128, CH], f32, name="ysb", tag="ysb")
                    nc.vector.tensor_add(
                        ysb[:], yps[:],
                        yacc[:, m, CH * nchk:CH * (nchk + 1)])
                    nc.sync.dma_start(
                        y[SUB * m:SUB * (m + 1),
                          CH * nchk:CH * (nchk + 1)],
                        ysb[:])
    nc.compile()
    return nc


def _prep_in_maps(embd_q, Wq, bq, Wk, bk, Wv, bv, Wo, bo):
    import ml_dtypes
    bf16 = ml_dtypes.bfloat16
    x = embd_q.reshape(NTOK, E).astype(np.float32)
    xT = np.ascontiguousarray(x.T.astype(bf16))
    eye = np.eye(128, dtype=bf16)
    r = np.arange(128)
    # pT is [k-part, q-col]; mask out k > q (future tokens)
    tri01 = np.ascontiguousarray(
        np.where(r[:, None] > r[None, :], 0.0, 1.0).astype(bf16))
    ones_v = np.ones((128, NCH * NSUB), dtype=bf16)
    bo_b = np.ascontiguousarray(
        np.broadcast_to(bo.astype(np.float32), (128, E)))
    woTf = Wo.astype(np.float32).T  # [feat, out]
    # pair-interleaved per-head layouts: partition q of pair p maps to
    # feat = 128*(2p) + q  (q < 64, even kt)  or  128*(2p+1) + (q-64)
    idx = np.zeros((4, 128), dtype=np.int64)
    for p in range(4):
        idx[p, :64] = 128 * (2 * p) + np.arange(64)
        idx[p, 64:] = 128 * (2 * p + 1) + np.arange(64)
    woh0 = np.ascontiguousarray(woTf[idx.reshape(-1)].astype(bf16))
    woh1 = np.ascontiguousarray(woTf[(idx + 64).reshape(-1)].astype(bf16))
    in_maps = []
    for c in range(NCORE):
        sl = slice(128 * c, 128 * (c + 1))
        in_maps.append({
            "xT": xT,
            "wqT": np.ascontiguousarray(Wq[sl].astype(np.float32).T.astype(bf16)),
            "wkT": np.ascontiguousarray(Wk[sl].astype(np.float32).T.astype(bf16)),
            "wvT": np.ascontiguousarray(Wv[sl].astype(np.float32).T.astype(bf16)),
            "woh0": woh0,
            "woh1": woh1,
            "bqs": np.ascontiguousarray(
                (bq[sl] * 0.125).reshape(128, 1), dtype=np.float32),
            "bks": np.ascontiguousarray(bk[sl].reshape(128, 1),
                                        dtype=np.float32),
            "bvs": np.ascontiguousarray(bv[sl].reshape(128, 1),
                                        dtype=np.float32),
            "bo_b": bo_b,
            "eye": eye,
            "tri01": tri01,
            "ones_v": ones_v,
        })
    return in_maps


def kernel(embd_q, Wq, bq, Wk, bk, Wv, bv, Wo, bo, _trace=False,
           _debug=False):
    key = ("dbg" if _debug else "nc")
    if key not in _nc_cache:
        _nc_cache[key] = build_nc(_debug=_debug)
    in_maps = _prep_in_maps(np.asarray(embd_q), np.asarray(Wq), np.asarray(bq),
                            np.asarray(Wk), np.asarray(bk), np.asarray(Wv),
                            np.asarray(bv), np.asarray(Wo), np.asarray(bo))
    import os
    tc_env = os.environ.get("TRACE_CORES")
    res = run_bass_kernel_spmd(
        _nc_cache[key], in_maps, list(range(NCORE)), trace=_trace,
        trace_cores=(list(range(NCORE)) if tc_env else None))
    out = np.concatenate(
        [res.results[c]["y"] for c in range(NCORE)], axis=0)
    out = out.reshape(B, T, E)
    kernel.last_results = res
    return out
